# revision 28
# baseline (speedup 1.0000x reference)
"""Bayesian-router MoE kernel for 8 Trainium2 NeuronCores.

Strategy (expert-parallel, per sharding hint):
  - Router moments / top-k / combine weights: tiny (B*F*E ~ 17 MFLOP), computed
    on host in float64 (min rank4/rank5 score gap is ~1.7e-4, far above fp32
    noise, so expert selection is stable vs the fp32 reference).
  - Token dispatch: host gathers each expert's routed tokens into a padded,
    transposed buffer XgT [P, FK, CAP] (the host-side equivalent of the
    all-to-all; full I/O contract means shard/unshard happens on host).
    Experts are sorted by token count: the 8 largest go to slot 0 (cap0),
    the 8 smallest to slot 1 (cap1 <= cap0), one of each per core, so the
    SPMD program wastes less padding compute.
  - Device: each of the 8 cores runs its 2-expert MLP on gathered tokens,
    entirely in transposed form (A1T = relu(W1^T XgT + b1), YT = W2^T A1T + b2)
    so no on-device transposes are needed.  Weights for one slot are packed
    on host into a single [P, FK*H + HK*C] fp16 tensor whose column layout
    exactly matches the lhsT tiles the PE consumes ([m][k][128] blocks), so
    weight DMA is a handful of large fully-contiguous transfers.  Everything
    ships and multiplies as fp16 (PSUM accumulation stays fp32): half the
    DMA vs fp32 and 1 cycle/row on the PE.
  - Ramp: slot-0 weights stream in first-use order on the sync HWDGE ring
    (m0-1, bias, m2-4, m5-7, then the gated L2 block) while the token
    chunks ride the scalar ring, so the PE starts as soon as ~550 KB has
    landed and never stalls mid-stream.  Bulk transfers (slot-0 L2, all of
    slot 1) are gated behind early slot-0 evictions so the SDMA round-robin
    doesn't steal ramp bandwidth from the critical pieces.
  - No SWDGE (gpsimd) DMA anywhere: keeping the Pool engine's DMA ring
    idle removes its drain/reset work from the end-of-program sequence
    (~1.5 us off the measured window).
  - Outputs ship back as fp16 (rounding ~3e-4, well inside tolerance),
    halving the output DMA; the final m-group of the last slot evicts in
    two half-size chunks split across both eviction engines so the
    kernel's last DMA covers only 256 columns and completes sooner.
  - Combine: host scatter-adds w[t,e] * Y_e rows into the output (the
    cross-device reduction of the unshard step).
"""

import os
import numpy as np

NCORES = 8
P = 128
TOP_K = 4


# ---------------------------------------------------------------------------
# host-side routing (matches reference math; float64 for stable ordering)
# ---------------------------------------------------------------------------
def _routing(h, W_mu, b_mu, W_logvar, b_logvar):
    h64 = h.astype(np.float64)
    mu = h64 @ W_mu.T.astype(np.float64) + b_mu.astype(np.float64)
    var = (h64 * h64) @ np.exp(W_logvar.astype(np.float64)).T + np.exp(
        b_logvar.astype(np.float64)
    )
    var = np.maximum(var, 1e-12)
    tilde = mu / np.sqrt(1.0 + (np.pi / 8.0) * var)
    t = tilde - tilde.max(axis=1, keepdims=True)
    ex = np.exp(t)
    probs = ex / ex.sum(axis=1, keepdims=True)
    idx = np.argsort(-tilde, axis=1, kind="stable")[:, :TOP_K]
    w = np.take_along_axis(probs, idx, axis=1)
    w = w / np.maximum(w.sum(axis=1, keepdims=True), 1e-12)
    return idx, w


def _chunks(cap):
    n = (cap + 511) // 512
    base, rem = divmod(cap, n)
    out = []
    off = 0
    for i in range(n):
        sz = base + (1 if i < rem else 0)
        out.append((off, sz))
        off += sz
    return out


# ---------------------------------------------------------------------------
# device kernel: 2-expert MLP on pre-gathered transposed tokens
# ---------------------------------------------------------------------------
def _build_kernel(F, H, C, caps):
    import concourse.mybir as mybir
    import concourse.tile as tile
    from concourse import bacc

    f32 = mybir.dt.float32
    f16 = mybir.dt.float16
    FK, HK, CK = F // P, H // P, C // P
    W1COLS = HK * FK * P          # L1 lhsT block: [m][k][128]
    W2COLS = CK * HK * P          # L2 lhsT block: [m][k][128]
    nslots = len(caps)

    nc = bacc.Bacc("TRN2", target_bir_lowering=False, debug=False,
                   num_devices=NCORES)

    xts_d = [nc.dram_tensor(f"xt{s}", [P, FK, caps[s]], f16,
                            kind="ExternalInput")
             for s in range(nslots)]
    yts_d = [nc.dram_tensor(f"yt{s}", [P, CK, caps[s]], f16,
                            kind="ExternalOutput")
             for s in range(nslots)]
    wpk = nc.dram_tensor("w", [nslots, P, W1COLS + W2COLS], f16,
                         kind="ExternalInput")
    bias = nc.dram_tensor("b", [P, nslots, HK + CK], f32,
                          kind="ExternalInput")

    with tile.TileContext(nc) as tc:
        with (
            tc.tile_pool(name="consts", bufs=1) as consts,
            tc.tile_pool(name="wpool", bufs=2) as wpool,
            tc.tile_pool(name="xpool", bufs=2) as xpool,
            tc.tile_pool(name="apool", bufs=2) as apool,
            tc.tile_pool(name="ypool", bufs=2) as ypool,
            tc.tile_pool(name="psum", bufs=8, space="PSUM") as pp,
        ):
            # bias is tiny (12 KB) and rides the sync HWDGE ring right
            # behind the first weight piece -- using no SWDGE (gpsimd) DMA
            # at all keeps the Pool engine's DMA ring out of the kernel's
            # end-of-program drain/reset sequence
            bs = consts.tile([P, nslots, HK + CK], f32)

            add, amax = mybir.AluOpType.add, mybir.AluOpType.max

            def evict(i, dst, src, bias_ap, relu):
                # alternate PSUM evictions between Scalar(ACT) and Vector(DVE)
                # so neither engine falls behind the matmul stream
                if i % 2 == 0:
                    return nc.scalar.activation(
                        dst, src,
                        mybir.ActivationFunctionType.Relu if relu
                        else mybir.ActivationFunctionType.Identity,
                        bias=bias_ap,
                    )
                elif relu:
                    return nc.vector.tensor_scalar(dst, src, bias_ap, 0.0,
                                                   add, amax)
                else:
                    return nc.vector.tensor_scalar_add(dst, src, bias_ap)

            # SDMA engines round-robin across queued transfers, so a
            # later-needed bulk transfer queued early steals bandwidth from
            # (and delays the completion of) the ramp-critical pieces of the
            # first slot. gate_after delays those transfers behind an early
            # eviction of the preceding phase.
            l1_evs = []

            def gate_after(dma_binst, gate_inst):
                if gate_inst is not None:
                    tile.add_dep_helper(
                        dma_binst.ins, gate_inst.ins,
                        reason="delay bulk DMA past the ramp-critical phase",
                    )

            for s in range(nslots):
                cap = caps[s]
                chunks = _chunks(cap)
                rchunks = chunks

                xts = xpool.tile([P, FK, cap], f16, tag=f"xt{s}")
                wt = wpool.tile([P, W1COLS + W2COLS], f16, tag="w")
                if s == 0:
                    # ramp-critical: first token chunk + first m-group's
                    # weights land first, issued in parallel on the two
                    # HWDGE rings (FIFO per ring, so ordering pieces by
                    # first-use keeps the PE fed); the bulky L2 piece is
                    # gated behind an early eviction so it doesn't
                    # round-robin-steal ramp bandwidth.
                    xcuts = [0] + [c for c, _ in rchunks[1:]] + [cap]
                    for c0, c1 in zip(xcuts[:-1], xcuts[1:]):
                        nc.scalar.dma_start(out=xts[:, :, c0:c1],
                                            in_=xts_d[s][:, :, c0:c1])
                    MG = FK * P  # one L1 m-group of lhsT columns
                    wcuts = [0, 2 * MG, 5 * MG, W1COLS]
                    for i, (c0, c1) in enumerate(zip(wcuts[:-1], wcuts[1:])):
                        nc.sync.dma_start(out=wt[:, c0:c1],
                                          in_=wpk[s][:, c0:c1])
                        if i == 0:
                            nc.sync.dma_start(out=bs[:], in_=bias[:])
                    w2_dma = nc.sync.dma_start(
                        out=wt[:, W1COLS:], in_=wpk[s][:, W1COLS:])
                else:
                    gate_after(
                        nc.scalar.dma_start(out=xts[:], in_=xts_d[s][:]),
                        l1_evs[2] if len(l1_evs) > 2 else None,
                    )
                    # split this slot's weights across BOTH HWDGE rings:
                    # halving the transfer's wall time halves the window in
                    # which DMA writes contend with PE operand reads (the
                    # dominant source of mid-stream matmul stall)
                    gate_after(
                        nc.sync.dma_start(out=wt[:, :W1COLS],
                                          in_=wpk[s][:, :W1COLS]),
                        l1_evs[3] if len(l1_evs) > 3 else None,
                    )
                    gate_after(
                        nc.scalar.dma_start(out=wt[:, W1COLS:],
                                            in_=wpk[s][:, W1COLS:]),
                        l1_evs[3] if len(l1_evs) > 3 else None,
                    )

                a1s = apool.tile([P, HK, cap], f16, tag="a1")
                ysb = ypool.tile([P, CK, cap], f16, tag="yt")

                # all layer-1 m-groups first, then all layer-2: PE has ready
                # work across the L1->L2 boundary and the slot seam
                ev = 0
                for m in range(HK):
                    for n0, nsz in chunks:
                        ps = pp.tile([P, 512], f32, tag="ps")
                        for k in range(FK):
                            w0 = m * (FK * P) + k * P
                            nc.tensor.matmul(
                                ps[:, :nsz],
                                wt[:, w0:w0 + P],
                                xts[:, k, n0:n0 + nsz],
                                start=(k == 0),
                                stop=(k == FK - 1),
                            )
                        e_inst = evict(ev, a1s[:, m, n0:n0 + nsz],
                                       ps[:, :nsz], bs[:, s, m:m + 1],
                                       relu=True)
                        if s == 0:
                            l1_evs.append(e_inst)
                        ev += 1
                if s == 0:
                    # release slot-0 L2 weights once layer 1 is underway
                    gate_after(w2_dma, l1_evs[1])

                last = (s == nslots - 1)
                for m in range(CK):
                    # final m-group of the final slot: halve the chunks so
                    # the tail eviction+DMA covers fewer columns and the
                    # kernel's last DMA completes sooner
                    mchunks = chunks
                    if last and m == CK - 1 and len(chunks) == 1 \
                            and cap % 2 == 0:
                        mchunks = [(0, cap // 2), (cap // 2, cap // 2)]
                    for n0, nsz in mchunks:
                        ps = pp.tile([P, 512], f32, tag="ps")
                        for k in range(HK):
                            w0 = W1COLS + m * (HK * P) + k * P
                            nc.tensor.matmul(
                                ps[:, :nsz],
                                wt[:, w0:w0 + P],
                                a1s[:, k, n0:n0 + nsz],
                                start=(k == 0),
                                stop=(k == HK - 1),
                            )
                        if last and m == CK - 1 and n0 + nsz == cap:
                            # final eviction: split across both engines in
                            # parallel so the last output DMA starts sooner
                            half = nsz // 2
                            evict(0, ysb[:, m, n0:n0 + half],
                                  ps[:, :half], bs[:, s, HK + m:HK + m + 1],
                                  relu=False)
                            evict(1, ysb[:, m, n0 + half:n0 + nsz],
                                  ps[:, half:nsz],
                                  bs[:, s, HK + m:HK + m + 1],
                                  relu=False)
                        else:
                            evict(ev, ysb[:, m, n0:n0 + nsz],
                                  ps[:, :nsz], bs[:, s, HK + m:HK + m + 1],
                                  relu=False)
                        ev += 1
                        if last and m == CK - 1:
                            # stream each tail chunk as soon as it's evicted
                            nc.sync.dma_start(
                                out=yts_d[s][:, m, n0:n0 + nsz],
                                in_=ysb[:, m, n0:n0 + nsz])
                        elif n0 + nsz == cap:
                            # whole row of ysb done -> stream it out on the
                            # sync ring (idle once weights are in)
                            nc.sync.dma_start(out=yts_d[s][:, m],
                                              in_=ysb[:, m])

    nc.compile()
    return nc


# ---------------------------------------------------------------------------
# entry point
# ---------------------------------------------------------------------------
def kernel(h, W_mu, b_mu, W_logvar, b_logvar, W1, b1, W2, b2):
    from concourse.bass_utils import run_bass_kernel_spmd

    h = np.ascontiguousarray(np.asarray(h, dtype=np.float32))
    W1 = np.asarray(W1, dtype=np.float32)
    b1 = np.asarray(b1, dtype=np.float32)
    W2 = np.asarray(W2, dtype=np.float32)
    b2 = np.asarray(b2, dtype=np.float32)

    B, F = h.shape
    E, _, H = W1.shape
    C = W2.shape[2]
    assert E % NCORES == 0
    nslots = E // NCORES
    FK, HK, CK = F // P, H // P, C // P
    W1COLS, W2COLS = HK * FK * P, CK * HK * P

    topk_idx, topk_w = _routing(
        np.asarray(h), np.asarray(W_mu), np.asarray(b_mu),
        np.asarray(W_logvar), np.asarray(b_logvar)
    )

    # per-expert token lists; sort experts by count so each slot's capacity
    # is the max within that slot (slot 0 = busiest experts)
    toks, poss = [], []
    counts = np.zeros(E, np.int64)
    for e in range(E):
        tok, pos = np.nonzero(topk_idx == e)
        toks.append(tok)
        poss.append(pos)
        counts[e] = len(tok)
    perm = np.argsort(-counts, kind="stable")
    caps = []
    for s in range(nslots):
        grp = perm[s * NCORES:(s + 1) * NCORES]
        caps.append(max(64, int(-(-counts[grp].max() // 32) * 32)))

    # gather/dispatch: tokens as [P, FK, cap] per expert (PE-ready layout);
    # weights packed per slot into one [P, W1COLS+W2COLS] lhsT-layout tensor
    xt = [np.zeros((NCORES, P, FK, caps[s]), np.float16)
          for s in range(nslots)]
    w_in = np.empty((NCORES, nslots, P, W1COLS + W2COLS), np.float16)
    b_in = np.empty((NCORES, P, nslots, HK + CK), np.float32)
    for i, e in enumerate(perm):
        s, c = divmod(i, NCORES)
        cnt = counts[e]
        xt[s][c, :, :, :cnt] = (
            h[toks[e]].T.astype(np.float16).reshape(FK, P, cnt)
            .transpose(1, 0, 2)
        )
        w_in[c, s, :, :W1COLS] = (
            W1[e].astype(np.float16).reshape(FK, P, HK, P)
            .transpose(1, 2, 0, 3).reshape(P, W1COLS)
        )
        w_in[c, s, :, W1COLS:] = (
            W2[e].astype(np.float16).reshape(HK, P, CK, P)
            .transpose(1, 2, 0, 3).reshape(P, W2COLS)
        )
        b_in[c, :, s, :HK] = b1[e].reshape(HK, P).T
        b_in[c, :, s, HK:] = b2[e].reshape(CK, P).T

    nc = _build_kernel(F, H, C, caps)

    in_maps = []
    for c in range(NCORES):
        m = {"w": w_in[c], "b": b_in[c]}
        for s in range(nslots):
            m[f"xt{s}"] = xt[s][c]
        in_maps.append(m)

    trace = bool(os.environ.get("MOE_KERNEL_TRACE"))
    res = run_bass_kernel_spmd(nc, in_maps, list(range(NCORES)), trace=trace)
    global LAST_RESULTS
    LAST_RESULTS = res

    # combine: scatter-add weighted expert outputs
    out = np.zeros((B, C), np.float32)
    for i, e in enumerate(perm):
        s, c = divmod(i, NCORES)
        cnt = counts[e]
        yte = res.results[c][f"yt{s}"]  # [P, CK, cap_s] fp16
        ye = yte.transpose(1, 0, 2).reshape(C, caps[s])[:, :cnt]
        out[toks[e]] += (
            topk_w[toks[e], poss[e]].astype(np.float32)[:, None]
            * ye.T.astype(np.float32)
        )
    return out


LAST_RESULTS = None


# revision 31
# speedup vs baseline: 1.1850x; 1.1850x over previous
"""Bayesian-router MoE kernel for 8 Trainium2 NeuronCores.

Strategy (expert-parallel, per sharding hint):
  - Router moments / top-k / combine weights: tiny (B*F*E ~ 17 MFLOP), computed
    on host in float64 (min rank4/rank5 score gap is ~1.7e-4, far above fp32
    noise, so expert selection is stable vs the fp32 reference).
  - Token dispatch: host gathers each expert's routed tokens into a padded,
    transposed buffer XgT [P, FK, CAP] (the host-side equivalent of the
    all-to-all; full I/O contract means shard/unshard happens on host).
    Experts are sorted by token count: the 8 largest go to slot 0 (cap0),
    the 8 smallest to slot 1 (cap1 <= cap0), one of each per core, so the
    SPMD program wastes less padding compute.
  - Device: each of the 8 cores runs its 2-expert MLP on gathered tokens,
    entirely in transposed form (A1T = relu(W1^T XgT + b1), YT = W2^T A1T + b2)
    so no on-device transposes are needed.  Weights for one slot are packed
    on host into a single [P, FK*H + HK*C] fp16 tensor whose column layout
    exactly matches the lhsT tiles the PE consumes ([m][k][128] blocks), so
    weight DMA is a handful of large fully-contiguous transfers.  Everything
    ships and multiplies as fp16 (PSUM accumulation stays fp32): half the
    DMA vs fp32 and 1 cycle/row on the PE.
  - Ramp: slot-0 weights stream in first-use order on the sync HWDGE ring
    (m0-1, bias, m2-4, m5-7, then the gated L2 block) while the token
    chunks ride the scalar ring, so the PE starts as soon as ~550 KB has
    landed and never stalls mid-stream.  Bulk transfers (slot-0 L2, all of
    slot 1) are gated behind early slot-0 evictions so the SDMA round-robin
    doesn't steal ramp bandwidth from the critical pieces.
  - No SWDGE (gpsimd) DMA anywhere: keeping the Pool engine's DMA ring
    idle removes its drain/reset work from the end-of-program sequence
    (~1.5 us off the measured window).
  - Outputs ship back as fp16 (rounding ~3e-4, well inside tolerance),
    halving the output DMA; the final m-group of the last slot evicts in
    two half-size chunks split across both eviction engines so the
    kernel's last DMA covers only 256 columns and completes sooner.
  - Combine: host scatter-adds w[t,e] * Y_e rows into the output (the
    cross-device reduction of the unshard step).
"""

import os
import numpy as np

NCORES = 8
P = 128
TOP_K = 4


# ---------------------------------------------------------------------------
# host-side routing (matches reference math; float64 for stable ordering)
# ---------------------------------------------------------------------------
def _routing(h, W_mu, b_mu, W_logvar, b_logvar):
    h64 = h.astype(np.float64)
    mu = h64 @ W_mu.T.astype(np.float64) + b_mu.astype(np.float64)
    var = (h64 * h64) @ np.exp(W_logvar.astype(np.float64)).T + np.exp(
        b_logvar.astype(np.float64)
    )
    var = np.maximum(var, 1e-12)
    tilde = mu / np.sqrt(1.0 + (np.pi / 8.0) * var)
    t = tilde - tilde.max(axis=1, keepdims=True)
    ex = np.exp(t)
    probs = ex / ex.sum(axis=1, keepdims=True)
    idx = np.argsort(-tilde, axis=1, kind="stable")[:, :TOP_K]
    w = np.take_along_axis(probs, idx, axis=1)
    w = w / np.maximum(w.sum(axis=1, keepdims=True), 1e-12)
    return idx, w


def _chunks(cap):
    n = (cap + 511) // 512
    base, rem = divmod(cap, n)
    out = []
    off = 0
    for i in range(n):
        sz = base + (1 if i < rem else 0)
        out.append((off, sz))
        off += sz
    return out


# ---------------------------------------------------------------------------
# device kernel: 2-expert MLP on pre-gathered transposed tokens
# ---------------------------------------------------------------------------
def _build_kernel(F, H, C, caps):
    import concourse.mybir as mybir
    import concourse.tile as tile
    from concourse import bacc

    f32 = mybir.dt.float32
    f16 = mybir.dt.float16
    FK, HK, CK = F // P, H // P, C // P
    W1COLS = HK * FK * P          # L1 lhsT block: [m][k][128]
    W2COLS = CK * HK * P          # L2 lhsT block: [m][k][128]
    nslots = len(caps)

    nc = bacc.Bacc("TRN2", target_bir_lowering=False, debug=False,
                   num_devices=NCORES)

    xts_d = [nc.dram_tensor(f"xt{s}", [P, FK, caps[s]], f16,
                            kind="ExternalInput")
             for s in range(nslots)]
    yts_d = [nc.dram_tensor(f"yt{s}", [P, CK, caps[s]], f16,
                            kind="ExternalOutput")
             for s in range(nslots)]
    wpk = nc.dram_tensor("w", [nslots, P, W1COLS + W2COLS], f16,
                         kind="ExternalInput")
    bias = nc.dram_tensor("b", [P, nslots, HK + CK], f32,
                          kind="ExternalInput")

    with tile.TileContext(nc) as tc:
        with (
            tc.tile_pool(name="consts", bufs=1) as consts,
            tc.tile_pool(name="wpool", bufs=2) as wpool,
            tc.tile_pool(name="xpool", bufs=2) as xpool,
            tc.tile_pool(name="apool", bufs=2) as apool,
            tc.tile_pool(name="ypool", bufs=2) as ypool,
            tc.tile_pool(name="psum", bufs=8, space="PSUM") as pp,
        ):
            # bias is tiny (12 KB) and rides the sync HWDGE ring right
            # behind the first weight piece -- using no SWDGE (gpsimd) DMA
            # at all keeps the Pool engine's DMA ring out of the kernel's
            # end-of-program drain/reset sequence
            bs = consts.tile([P, nslots, HK + CK], f32)

            add, amax = mybir.AluOpType.add, mybir.AluOpType.max

            def evict(i, dst, src, bias_ap, relu):
                # alternate PSUM evictions between Scalar(ACT) and Vector(DVE)
                # so neither engine falls behind the matmul stream
                if i % 2 == 0:
                    return nc.scalar.activation(
                        dst, src,
                        mybir.ActivationFunctionType.Relu if relu
                        else mybir.ActivationFunctionType.Identity,
                        bias=bias_ap,
                    )
                elif relu:
                    return nc.vector.tensor_scalar(dst, src, bias_ap, 0.0,
                                                   add, amax)
                else:
                    return nc.vector.tensor_scalar_add(dst, src, bias_ap)

            # SDMA engines round-robin across queued transfers, so a
            # later-needed bulk transfer queued early steals bandwidth from
            # (and delays the completion of) the ramp-critical pieces of the
            # first slot. gate_after delays those transfers behind an early
            # eviction of the preceding phase.  Slot-1 inputs not needed
            # until its own L1 (tokens, L2 weights) are pushed all the way
            # past slot-0's L1 into the L2 window, where concurrent DMA
            # writes stall the PE far less.
            l1_evs = []
            l2_evs = []

            def gate_after(dma_binst, gate_inst):
                if gate_inst is not None:
                    tile.add_dep_helper(
                        dma_binst.ins, gate_inst.ins,
                        reason="delay bulk DMA past the ramp-critical phase",
                    )

            for s in range(nslots):
                cap = caps[s]
                chunks = _chunks(cap)
                rchunks = chunks

                xts = xpool.tile([P, FK, cap], f16, tag=f"xt{s}")
                wt = wpool.tile([P, W1COLS + W2COLS], f16, tag="w")
                if s == 0:
                    # ramp-critical: first token chunk + first m-group's
                    # weights land first, issued in parallel on the two
                    # HWDGE rings (FIFO per ring, so ordering pieces by
                    # first-use keeps the PE fed); the bulky L2 piece is
                    # gated behind an early eviction so it doesn't
                    # round-robin-steal ramp bandwidth.
                    xcuts = [0] + [c for c, _ in rchunks[1:]] + [cap]
                    for c0, c1 in zip(xcuts[:-1], xcuts[1:]):
                        nc.scalar.dma_start(out=xts[:, :, c0:c1],
                                            in_=xts_d[s][:, :, c0:c1])
                    MG = FK * P  # one L1 m-group of lhsT columns
                    wcuts = [0, 2 * MG, 5 * MG, W1COLS]
                    for i, (c0, c1) in enumerate(zip(wcuts[:-1], wcuts[1:])):
                        nc.sync.dma_start(out=wt[:, c0:c1],
                                          in_=wpk[s][:, c0:c1])
                        if i == 0:
                            nc.sync.dma_start(out=bs[:], in_=bias[:])
                    w2_dma = nc.sync.dma_start(
                        out=wt[:, W1COLS:], in_=wpk[s][:, W1COLS:])
                else:
                    gate_after(
                        nc.scalar.dma_start(out=xts[:], in_=xts_d[s][:]),
                        l2_evs[0] if l2_evs else None,
                    )
                    # split this slot's weights across BOTH HWDGE rings:
                    # halving the transfer's wall time halves the window in
                    # which DMA writes contend with PE operand reads (the
                    # dominant source of mid-stream matmul stall); the L1
                    # half is needed first and releases early, the L2 half
                    # waits for slot-0's L2 window
                    gate_after(
                        nc.sync.dma_start(out=wt[:, :W1COLS],
                                          in_=wpk[s][:, :W1COLS]),
                        l1_evs[3] if len(l1_evs) > 3 else None,
                    )
                    gate_after(
                        nc.scalar.dma_start(out=wt[:, W1COLS:],
                                            in_=wpk[s][:, W1COLS:]),
                        l2_evs[0] if l2_evs else None,
                    )

                a1s = apool.tile([P, HK, cap], f16, tag="a1")
                ysb = ypool.tile([P, CK, cap], f16, tag="yt")

                # all layer-1 m-groups first, then all layer-2: PE has ready
                # work across the L1->L2 boundary and the slot seam
                ev = 0
                for m in range(HK):
                    for n0, nsz in chunks:
                        ps = pp.tile([P, 512], f32, tag="ps")
                        for k in range(FK):
                            w0 = m * (FK * P) + k * P
                            nc.tensor.matmul(
                                ps[:, :nsz],
                                wt[:, w0:w0 + P],
                                xts[:, k, n0:n0 + nsz],
                                start=(k == 0),
                                stop=(k == FK - 1),
                            )
                        e_inst = evict(ev, a1s[:, m, n0:n0 + nsz],
                                       ps[:, :nsz], bs[:, s, m:m + 1],
                                       relu=True)
                        if s == 0:
                            l1_evs.append(e_inst)
                        ev += 1
                if s == 0:
                    # release slot-0 L2 weights once layer 1 is underway
                    gate_after(w2_dma, l1_evs[1])

                last = (s == nslots - 1)
                for m in range(CK):
                    # final m-group of the final slot: halve the chunks so
                    # the tail eviction+DMA covers fewer columns and the
                    # kernel's last DMA completes sooner
                    mchunks = chunks
                    if last and m == CK - 1 and len(chunks) == 1 \
                            and cap % 2 == 0:
                        mchunks = [(0, cap // 2), (cap // 2, cap // 2)]
                    for n0, nsz in mchunks:
                        ps = pp.tile([P, 512], f32, tag="ps")
                        for k in range(HK):
                            w0 = W1COLS + m * (HK * P) + k * P
                            nc.tensor.matmul(
                                ps[:, :nsz],
                                wt[:, w0:w0 + P],
                                a1s[:, k, n0:n0 + nsz],
                                start=(k == 0),
                                stop=(k == HK - 1),
                            )
                        if last and m == CK - 1 and n0 + nsz == cap:
                            # final eviction: split across both engines in
                            # parallel so the last output DMA starts sooner
                            half = nsz // 2
                            evict(0, ysb[:, m, n0:n0 + half],
                                  ps[:, :half], bs[:, s, HK + m:HK + m + 1],
                                  relu=False)
                            evict(1, ysb[:, m, n0 + half:n0 + nsz],
                                  ps[:, half:nsz],
                                  bs[:, s, HK + m:HK + m + 1],
                                  relu=False)
                        else:
                            e2 = evict(ev, ysb[:, m, n0:n0 + nsz],
                                       ps[:, :nsz],
                                       bs[:, s, HK + m:HK + m + 1],
                                       relu=False)
                            if s == 0:
                                l2_evs.append(e2)
                        ev += 1
                        if last and m == CK - 1:
                            # stream each tail chunk as soon as it's evicted
                            nc.sync.dma_start(
                                out=yts_d[s][:, m, n0:n0 + nsz],
                                in_=ysb[:, m, n0:n0 + nsz])
                        elif n0 + nsz == cap:
                            # whole row of ysb done -> stream it out on the
                            # sync ring (idle once weights are in)
                            nc.sync.dma_start(out=yts_d[s][:, m],
                                              in_=ysb[:, m])

    nc.compile()
    return nc


# ---------------------------------------------------------------------------
# entry point
# ---------------------------------------------------------------------------
def kernel(h, W_mu, b_mu, W_logvar, b_logvar, W1, b1, W2, b2):
    from concourse.bass_utils import run_bass_kernel_spmd

    h = np.ascontiguousarray(np.asarray(h, dtype=np.float32))
    W1 = np.asarray(W1, dtype=np.float32)
    b1 = np.asarray(b1, dtype=np.float32)
    W2 = np.asarray(W2, dtype=np.float32)
    b2 = np.asarray(b2, dtype=np.float32)

    B, F = h.shape
    E, _, H = W1.shape
    C = W2.shape[2]
    assert E % NCORES == 0
    nslots = E // NCORES
    FK, HK, CK = F // P, H // P, C // P
    W1COLS, W2COLS = HK * FK * P, CK * HK * P

    topk_idx, topk_w = _routing(
        np.asarray(h), np.asarray(W_mu), np.asarray(b_mu),
        np.asarray(W_logvar), np.asarray(b_logvar)
    )

    # per-expert token lists; sort experts by count so each slot's capacity
    # is the max within that slot (slot 0 = busiest experts)
    toks, poss = [], []
    counts = np.zeros(E, np.int64)
    for e in range(E):
        tok, pos = np.nonzero(topk_idx == e)
        toks.append(tok)
        poss.append(pos)
        counts[e] = len(tok)
    perm = np.argsort(-counts, kind="stable")
    caps = []
    for s in range(nslots):
        grp = perm[s * NCORES:(s + 1) * NCORES]
        caps.append(max(64, int(-(-counts[grp].max() // 32) * 32)))

    # gather/dispatch: tokens as [P, FK, cap] per expert (PE-ready layout);
    # weights packed per slot into one [P, W1COLS+W2COLS] lhsT-layout tensor
    xt = [np.zeros((NCORES, P, FK, caps[s]), np.float16)
          for s in range(nslots)]
    w_in = np.empty((NCORES, nslots, P, W1COLS + W2COLS), np.float16)
    b_in = np.empty((NCORES, P, nslots, HK + CK), np.float32)
    for i, e in enumerate(perm):
        s, c = divmod(i, NCORES)
        cnt = counts[e]
        xt[s][c, :, :, :cnt] = (
            h[toks[e]].T.astype(np.float16).reshape(FK, P, cnt)
            .transpose(1, 0, 2)
        )
        w_in[c, s, :, :W1COLS] = (
            W1[e].astype(np.float16).reshape(FK, P, HK, P)
            .transpose(1, 2, 0, 3).reshape(P, W1COLS)
        )
        w_in[c, s, :, W1COLS:] = (
            W2[e].astype(np.float16).reshape(HK, P, CK, P)
            .transpose(1, 2, 0, 3).reshape(P, W2COLS)
        )
        b_in[c, :, s, :HK] = b1[e].reshape(HK, P).T
        b_in[c, :, s, HK:] = b2[e].reshape(CK, P).T

    nc = _build_kernel(F, H, C, caps)

    in_maps = []
    for c in range(NCORES):
        m = {"w": w_in[c], "b": b_in[c]}
        for s in range(nslots):
            m[f"xt{s}"] = xt[s][c]
        in_maps.append(m)

    trace = bool(os.environ.get("MOE_KERNEL_TRACE"))
    res = run_bass_kernel_spmd(nc, in_maps, list(range(NCORES)), trace=trace)
    global LAST_RESULTS
    LAST_RESULTS = res

    # combine: scatter-add weighted expert outputs
    out = np.zeros((B, C), np.float32)
    for i, e in enumerate(perm):
        s, c = divmod(i, NCORES)
        cnt = counts[e]
        yte = res.results[c][f"yt{s}"]  # [P, CK, cap_s] fp16
        ye = yte.transpose(1, 0, 2).reshape(C, caps[s])[:, :cnt]
        out[toks[e]] += (
            topk_w[toks[e], poss[e]].astype(np.float32)[:, None]
            * ye.T.astype(np.float32)
        )
    return out


LAST_RESULTS = None
